# revision 1
# baseline (speedup 1.0000x reference)
"""CrossAttention Trainium2 Bass kernel (8 NeuronCores).

Problem (fp32): x [4, 2048, 1024], y [4, 2048, 768]
  q = x@Wq + bq; k = y@Wk + bk; v = y@Wv + bv           (16 heads x d_head 64)
  out = softmax(q k^T / 8) v  reshaped, then @ Wo + bo  -> [4, 2048, 1024]

Sharding: 8 cores = 4 batches x 2 head-halves. Core c handles batch c//2 and
heads (c%2)*8 .. +8 (d-slice of 512) for the full sequence, producing a
partial output [2048, 1024] = attn_half @ Wo[d_slice, :] (+ bo on half 0).
Host sums the two partials per batch. No duplicated FLOPs, no collectives.

Per-core dataflow (matmuls in float32r = full-rate ~tf32 precision):
  qT[d_half 512, sq 2048]  = (Wq_half)^T @ x^T   (+bq, per-partition add)
  KT[d_half 512, sk 2048]  = (Wk_half)^T @ y^T   (+bk)
  v' [sk 2048, 8*65]       = y @ Wv_aug (+bv_aug); 65th col per head = 1.0
                             (ones column makes the AV matmul emit softmax
                              denominators for free)
  per (head pair, sq-macro 512), software-pipelined over 8 groups of 2 sk:
    scoresT half-tiles per head via row-packed K=64 matmuls (2 heads concur.)
    expT = ACT exp(scoresT / 8), [128,1024] calls (PSUM->SBUF f32r),
      double-buffered score PSUM so ACT streams continuously
    out'[65, 512] accumulates v'^T-matmuls over 16 sk tiles (row 64 = denom);
      AV of group g is emitted after scores/exp of group g+1 so the in-order
      PE stream never blocks the next exp
    attn = out'[0:64] * bcast(1/denom)  (K=1 ones matmul broadcast + DVE mul)
  out_partial[sq, 1024] = attn^T-matmul @ Wo_half (+bo via ones bias-fill)
"""

import numpy as np

import concourse.bass as bass
import concourse.mybir as mybir
import concourse.tile as tile
from concourse.bass_utils import run_bass_kernel_spmd

F32 = mybir.dt.float32
F32R = mybir.dt.float32r
AF = mybir.ActivationFunctionType
ALU = mybir.AluOpType

B, S, DE, DC = 4, 2048, 1024, 768
H, DH = 16, 64
HH = H // 2          # heads per core
DHALF = DE // 2      # 512, d-slice per core
NMT = DHALF // 128   # 4 head pairs
NKT_X = DE // 128    # 8 k-tiles for q projection
NKT_Y = DC // 128    # 6 k-tiles for k/v projections
NSK = S // 128       # 16 sk tiles
NMAC = S // 512      # 4 sq macros
VW = DH + 1          # 65 cols per head in v' (ones column at 64)
VTOT = HH * VW       # 520
SCALE = 1.0 / np.sqrt(DH)

_prog_cache = {}


def _split_sync_waits(nc):
    """This container's walrus accepts only 1 sync wait per instruction.
    Tile attaches one wait per producer proc. For every instruction with k>1
    waits, insert k-1 single-wait nops on the same engine right before it
    (equivalent semantics: the engine's stream waits serially)."""
    eng_map = {
        mybir.EngineType.PE: nc.tensor,
        mybir.EngineType.Activation: nc.scalar,
        mybir.EngineType.DVE: nc.vector,
        mybir.EngineType.Pool: nc.gpsimd,
        mybir.EngineType.SP: nc.sync,
    }
    for bb in nc.main_func.blocks:
        insts = bb.instructions
        fixes = []
        for idx, ins in enumerate(insts):
            si = ins.sync_info
            if si and si.on_wait and len(si.on_wait) > 1:
                fixes.append((idx, ins))
        for idx, ins in reversed(fixes):
            eng = eng_map.get(ins.engine)
            if eng is None:
                continue
            waits = list(ins.sync_info.on_wait)
            ins.sync_info.on_wait = [waits[-1]]
            nops = []
            for w in waits[:-1]:
                n = eng.nop(nofuse=True).ins
                for b2 in nc.main_func.blocks:
                    if b2.instructions and b2.instructions[-1] is n:
                        b2.instructions.pop()
                        break
                n.sync_info = mybir.SyncInfo(on_wait=[w], on_update=[])
                nops.append(n)
            for j, n in enumerate(nops):
                insts.insert(idx + j, n)
    return nc


def build_program(n_reps: int = 1, upto: str = 'E'):
    nc = bass.Bass()

    xT = nc.dram_tensor("xT", [DE, S], F32R, kind="ExternalInput")
    yT = nc.dram_tensor("yT", [DC, S], F32R, kind="ExternalInput")
    wq = nc.dram_tensor("wq", [DE, DHALF], F32R, kind="ExternalInput")
    wk = nc.dram_tensor("wk", [DC, DHALF], F32R, kind="ExternalInput")
    wv = nc.dram_tensor("wv", [DC, VTOT], F32R, kind="ExternalInput")
    wo = nc.dram_tensor("wo", [DHALF, DE], F32R, kind="ExternalInput")
    bqd = nc.dram_tensor("bq", [128, NMT], F32, kind="ExternalInput")
    bkd = nc.dram_tensor("bk", [128, NMT], F32, kind="ExternalInput")
    bvd = nc.dram_tensor("bv", [1, VTOT], F32R, kind="ExternalInput")
    bod = nc.dram_tensor("bo", [1, DE], F32R, kind="ExternalInput")
    onesd = nc.dram_tensor("ones", [1, 128], F32R, kind="ExternalInput")
    outd = nc.dram_tensor("out", [S, DE], F32, kind="ExternalOutput")

    from contextlib import ExitStack

    with tile.TileContext(nc) as tc:
      for _rep in range(n_reps):  # >1 only for timing (amortizes dispatch)
        with ExitStack() as ctx:
            pconst = ctx.enter_context(tc.tile_pool(name="const", bufs=1))
            ones_sb = pconst.tile([1, 128], F32R, name="ones")
            bv_sb = pconst.tile([1, VTOT], F32R, name="bv")
            bo_sb = pconst.tile([1, DE], F32R, name="bo")
            bq_sb = pconst.tile([128, NMT], F32, name="bq")
            bk_sb = pconst.tile([128, NMT], F32, name="bk")
            nc.sync.dma_start(ones_sb[:], onesd[:])
            nc.sync.dma_start(bv_sb[:], bvd[:])
            nc.sync.dma_start(bo_sb[:], bod[:])
            nc.sync.dma_start(bq_sb[:], bqd[:])
            nc.sync.dma_start(bk_sb[:], bkd[:])

            pqT = ctx.enter_context(tc.tile_pool(name="qT", bufs=NMT))
            qT = [pqT.tile([128, S], F32R, name="qT") for _ in range(NMT)]

            # ---- Phase A: qT = Wq^T @ xT (+bq) ----
            with (
                tc.tile_pool(name="xTp", bufs=NKT_X) as pxT,
                tc.tile_pool(name="wqp", bufs=NKT_X) as pwq,
                tc.tile_pool(name="psA", bufs=2, space="PSUM") as psA,
            ):
                xt = []
                wqt = []
                for kt in range(NKT_X):
                    t = pxT.tile([128, S], F32R, name="xt")
                    nc.sync.dma_start(t[:], xT[kt * 128 : (kt + 1) * 128, :])
                    xt.append(t)
                    t = pwq.tile([128, DHALF], F32R, name="wqt")
                    nc.sync.dma_start(t[:], wq[kt * 128 : (kt + 1) * 128, :])
                    wqt.append(t)
                for mt in range(NMT):
                    for nn in range(NMAC):
                        ps = psA.tile([128, 512], F32, name="psA")
                        for kt in range(NKT_X):
                            nc.tensor.matmul(
                                ps[:],
                                wqt[kt][:, mt * 128 : (mt + 1) * 128],
                                xt[kt][:, nn * 512 : (nn + 1) * 512],
                                start=(kt == 0),
                                stop=(kt == NKT_X - 1),
                            )
                        with nc.allow_low_precision(reason="f32r store"):
                            nc.vector.tensor_scalar(
                                qT[mt][:, nn * 512 : (nn + 1) * 512],
                                ps[:],
                                bq_sb[:, mt : mt + 1],
                                None,
                                ALU.add,
                            )

            # ---- Phases B+C: KT and v' from yT ----
            pKT = ctx.enter_context(tc.tile_pool(name="KT", bufs=NMT))
            KT = [pKT.tile([128, S], F32R, name="KT") for _ in range(NMT)]
            pv = ctx.enter_context(tc.tile_pool(name="v", bufs=NSK))
            vsb = [pv.tile([128, VTOT], F32R, name="v") for _ in range(NSK)]
            with tc.tile_pool(name="yTp", bufs=NKT_Y) as pyT:
                yt = []
                for kt in range(NKT_Y):
                    t = pyT.tile([128, S], F32R, name="yt")
                    nc.sync.dma_start(t[:], yT[kt * 128 : (kt + 1) * 128, :])
                    yt.append(t)
                with (
                    tc.tile_pool(name="wkp", bufs=NKT_Y) as pwk,
                    tc.tile_pool(name="psB", bufs=2, space="PSUM") as psB,
                ):
                    wkt = []
                    for kt in range(NKT_Y):
                        t = pwk.tile([128, DHALF], F32R, name="wkt")
                        nc.sync.dma_start(t[:], wk[kt * 128 : (kt + 1) * 128, :])
                        wkt.append(t)
                    for mt in range(NMT):
                        for nn in range(NMAC):
                            ps = psB.tile([128, 512], F32, name="psB")
                            for kt in range(NKT_Y):
                                nc.tensor.matmul(
                                    ps[:],
                                    wkt[kt][:, mt * 128 : (mt + 1) * 128],
                                    yt[kt][:, nn * 512 : (nn + 1) * 512],
                                    start=(kt == 0),
                                    stop=(kt == NKT_Y - 1),
                                )
                            with nc.allow_low_precision(reason="f32r store"):
                                nc.vector.tensor_scalar(
                                    KT[mt][:, nn * 512 : (nn + 1) * 512],
                                    ps[:],
                                    bk_sb[:, mt : mt + 1],
                                    None,
                                    ALU.add,
                                )
                with (
                    tc.tile_pool(name="wvp", bufs=NKT_Y) as pwv,
                    tc.tile_pool(name="psC", bufs=2, space="PSUM") as psC,
                ):
                    wvt = []
                    for kt in range(NKT_Y):
                        t = pwv.tile([128, VTOT], F32R, name="wvt")
                        nc.sync.dma_start(t[:], wv[kt * 128 : (kt + 1) * 128, :])
                        wvt.append(t)
                    for sk in range(NSK):
                        for nn2 in range(2):
                            lo, hi = nn2 * 260, nn2 * 260 + 260
                            ps = psC.tile([128, 260], F32, name="psC")
                            nc.tensor.matmul(
                                ps[:],
                                ones_sb[:, :128],
                                bv_sb[:, lo:hi],
                                start=True,
                                stop=False,
                            )
                            for kt in range(NKT_Y):
                                nc.tensor.matmul(
                                    ps[:],
                                    yt[kt][:, sk * 128 : (sk + 1) * 128],
                                    wvt[kt][:, lo:hi],
                                    start=False,
                                    stop=(kt == NKT_Y - 1),
                                )
                            with nc.allow_low_precision(reason="f32r store"):
                                nc.vector.tensor_copy(vsb[sk][:, lo:hi], ps[:])

            # ---- Wo prefetch (DMA overlaps attention) ----
            pattn = ctx.enter_context(tc.tile_pool(name="attn", bufs=NMT))
            attn = [pattn.tile([128, S], F32R, name="attn") for _ in range(NMT)]
            pwo = ctx.enter_context(tc.tile_pool(name="wop", bufs=NMT))
            wot = []
            for kt in range(NMT):
                t = pwo.tile([128, DE], F32R, name="wot")
                nc.sync.dma_start(t[:], wo[kt * 128 : (kt + 1) * 128, :])
                wot.append(t)

            # ---- Phase D: attention, software-pipelined ----
            if upto == 'C':
                nc.gpsimd.dma_start(outd[0:128, :], qT[0][:, 0:1024])
                continue
            with (
                tc.tile_pool(name="expp", bufs=4) as pexp,
                tc.tile_pool(name="normp", bufs=2) as pnorm,
                tc.tile_pool(name="scps", bufs=2, space="PSUM") as pssc,
                tc.tile_pool(name="avps", bufs=2, space="PSUM") as psav,
                tc.tile_pool(name="bcps", bufs=1, space="PSUM") as psbc,
            ):
                for p in range(NMT):
                    for mac in range(NMAC):
                        sq = mac * 512
                        outp = [
                            psav.tile([VW, 512], F32, name="avps") for _ in range(2)
                        ]
                        prev = None  # (exptA, exptB, g)
                        for g in range(NSK // 2):
                            exps = []
                            for j in range(2):
                                sc = pssc.tile([128, 1024], F32, name="scps")
                                for d2 in range(2):
                                    t = 2 * g + d2
                                    nc.tensor.matmul(
                                        sc[:, d2 * 512 : (d2 + 1) * 512],
                                        KT[p][
                                            j * 64 : j * 64 + 64,
                                            t * 128 : (t + 1) * 128,
                                        ],
                                        qT[p][j * 64 : j * 64 + 64, sq : sq + 512],
                                        start=True,
                                        stop=True,
                                    )
                                ex = pexp.tile([128, 1024], F32R, name="expt")
                                nc.scalar.activation(
                                    ex[:], sc[:], AF.Exp, scale=SCALE
                                )
                                exps.append(ex)
                            if prev is not None:
                                pa, pb, pg = prev
                                for j, ex in ((0, pa), (1, pb)):
                                    lh = 2 * p + j
                                    for d2 in range(2):
                                        t = 2 * pg + d2
                                        nc.tensor.matmul(
                                            outp[j][:],
                                            vsb[t][:, lh * VW : (lh + 1) * VW],
                                            ex[:, d2 * 512 : (d2 + 1) * 512],
                                            start=(t == 0),
                                            stop=(t == NSK - 1),
                                        )
                            prev = (exps[0], exps[1], g)
                        pa, pb, pg = prev
                        for j, ex in ((0, pa), (1, pb)):
                            lh = 2 * p + j
                            for d2 in range(2):
                                t = 2 * pg + d2
                                nc.tensor.matmul(
                                    outp[j][:],
                                    vsb[t][:, lh * VW : (lh + 1) * VW],
                                    ex[:, d2 * 512 : (d2 + 1) * 512],
                                    start=(t == 0),
                                    stop=(t == NSK - 1),
                                )
                        for j in range(2):
                            rd = pnorm.tile([1, 512], F32R, name="rd")
                            with nc.allow_low_precision(reason="recip f32r"):
                                nc.vector.reciprocal(rd[:], outp[j][64:65, :])
                            bc = psbc.tile([64, 512], F32, name="bcps")
                            nc.tensor.matmul(
                                bc[:], ones_sb[:, :64], rd[:], start=True, stop=True
                            )
                            bcs = pnorm.tile([64, 512], F32, name="bcs")
                            nc.vector.tensor_copy(bcs[:], bc[:])
                            if j == 0:
                                with nc.allow_low_precision(reason="f32r store"):
                                    nc.vector.tensor_mul(
                                        attn[p][0:64, sq : sq + 512],
                                        outp[j][0:64, :],
                                        bcs[:],
                                    )
                            else:
                                tmp = pnorm.tile([64, 512], F32R, name="tmpn")
                                with nc.allow_low_precision(reason="f32r store"):
                                    nc.vector.tensor_mul(
                                        tmp[:], outp[j][0:64, :], bcs[:]
                                    )
                                # DVE lanes cannot shift partitions; DMA moves
                                # the odd head's rows to partitions 64..127
                                nc.sync.dma_start(
                                    attn[p][64:128, sq : sq + 512], tmp[:]
                                )

            # ---- Phase E: out = attn^T-matmul @ Wo (+bo) ----
            if upto == 'D':
                nc.gpsimd.dma_start(outd[0:128, :], attn[0][:, 0:1024])
                continue
            with (
                tc.tile_pool(name="outp", bufs=3) as pout,
                tc.tile_pool(name="psE", bufs=2, space="PSUM") as psE,
            ):
                for sm in range(NSK):
                    osb = pout.tile([128, DE], F32, name="osb")
                    for nn in range(2):
                        lo = nn * 512
                        ps = psE.tile([128, 512], F32, name="psE")
                        nc.tensor.matmul(
                            ps[:],
                            ones_sb[:, :128],
                            bo_sb[:, lo : lo + 512],
                            start=True,
                            stop=False,
                        )
                        for kt in range(NMT):
                            nc.tensor.matmul(
                                ps[:],
                                attn[kt][:, sm * 128 : (sm + 1) * 128],
                                wot[kt][:, lo : lo + 512],
                                start=False,
                                stop=(kt == NMT - 1),
                            )
                        nc.vector.tensor_copy(osb[:, lo : lo + 512], ps[:])
                    nc.sync.dma_start(outd[sm * 128 : (sm + 1) * 128, :], osb[:])

    return _split_sync_waits(nc)


def _host_prep(x, y, Wq, bq, Wk, bk, Wv, bv, Wo, bo):
    x = np.asarray(x, dtype=np.float32)
    y = np.asarray(y, dtype=np.float32)
    Wq = np.asarray(Wq, dtype=np.float32)
    Wk = np.asarray(Wk, dtype=np.float32)
    Wv = np.asarray(Wv, dtype=np.float32)
    Wo = np.asarray(Wo, dtype=np.float32)
    bq = np.asarray(bq, dtype=np.float32)
    bk = np.asarray(bk, dtype=np.float32)
    bv = np.asarray(bv, dtype=np.float32)
    bo = np.asarray(bo, dtype=np.float32)
    ones = np.ones((1, 128), dtype=np.float32)
    in_maps = []
    for c in range(8):
        b, hh = c // 2, c % 2
        dlo = hh * DHALF
        wv_aug = np.zeros((DC, VTOT), dtype=np.float32)
        bv_aug = np.zeros((1, VTOT), dtype=np.float32)
        for lh in range(HH):
            gh = hh * HH + lh
            wv_aug[:, lh * VW : lh * VW + DH] = Wv[:, gh * DH : (gh + 1) * DH]
            bv_aug[0, lh * VW : lh * VW + DH] = bv[gh * DH : (gh + 1) * DH]
            bv_aug[0, lh * VW + DH] = 1.0
        in_maps.append(
            {
                "xT": np.ascontiguousarray(x[b].T),
                "yT": np.ascontiguousarray(y[b].T),
                "wq": np.ascontiguousarray(Wq[:, dlo : dlo + DHALF]),
                "wk": np.ascontiguousarray(Wk[:, dlo : dlo + DHALF]),
                "wv": wv_aug,
                "wo": np.ascontiguousarray(Wo[dlo : dlo + DHALF, :]),
                "bq": np.ascontiguousarray(
                    bq[dlo : dlo + DHALF].reshape(NMT, 128).T
                ),
                "bk": np.ascontiguousarray(
                    bk[dlo : dlo + DHALF].reshape(NMT, 128).T
                ),
                "bv": bv_aug,
                "bo": (bo if hh == 0 else np.zeros_like(bo)).reshape(1, DE),
                "ones": ones,
            }
        )
    return in_maps


def kernel(x, y, Wq, bq, Wk, bk, Wv, bv, Wo, bo, _results_out=None, _trace=False):
    if "nc" not in _prog_cache:
        _prog_cache["nc"] = build_program()
    nc = _prog_cache["nc"]
    in_maps = _host_prep(x, y, Wq, bq, Wk, bk, Wv, bv, Wo, bo)
    res = run_bass_kernel_spmd(nc, in_maps, core_ids=list(range(8)), trace=_trace)
    if _results_out is not None:
        _results_out.append(res)
    parts = [res.results[c]["out"] for c in range(8)]
    out = np.stack(
        [parts[2 * b].astype(np.float32) + parts[2 * b + 1] for b in range(B)]
    )
    return out

